# revision 2
# baseline (speedup 1.0000x reference)
"""Multi-head self-attention (B=4, N=2048, C=1024, H=16) on 8 Trainium2 cores.

Sharding: core = (batch b, head-group g) with b in 0..3, g in 0..1.
Each core computes, for its batch and its 8 heads:
    QKV projection -> per-head attention (S^T layout softmax) -> proj partial.
Host sums the two head-group partials per batch and adds b_proj.

v2 schedule: heads are processed in PAIRS (2u, 2u+1). The two S^T matmuls
of a pair use disjoint PE row-groups (k-dims at partitions 0-63 vs 64-127),
so the hardware runs them concurrently (~2x on the QK^T stage). Each
[128 keys, 1024] PSUM tile holds S^T for both heads (512 queries each) and
is consumed by ONE wide exp on ACT - the exp stream is the kernel's
critical path, so everything else (QKV projection groups, proj groups) is
chopped into small "filler quanta" interleaved between attention steps to
soak up the PE slack without stalling ACT.

Device-side layout choices (all transposes done on host, none on device):
  - x is shipped pre-transposed as xT [C, N] (+ a ones row for the V bias).
  - Q^T/K^T are produced as [c', n] tiles directly (lhsT = w_qk natural).
  - V is produced in natural [m, h*d] layout augmented with a ones column per
    head; the ones column makes the PV matmul emit the softmax row-sums.
  - Softmax runs on S^T tiles [m, n]: exp on the scalar engine, sums via the
    V ones-column, normalization via reciprocal + partition-broadcast + mult.
  - Projection consumes O^T [hd, n] tiles directly as lhsT.
"""

import os
import sys

if "/opt/trn_rl_repo" not in sys.path:
    sys.path.insert(0, "/opt/trn_rl_repo")

# the kernel executes through PJRT on the axon-tunneled NeuronCores; a
# cpu-pinned JAX_PLATFORMS (as some harnesses set for the reference) would
# hide the devices — fix it before anything imports jax
if "axon" not in os.environ.get("JAX_PLATFORMS", "axon"):
    os.environ["JAX_PLATFORMS"] = "axon"

from collections import deque
from contextlib import ExitStack

import ml_dtypes
import numpy as np

import concourse.bass as bass
import concourse.tile as tile
from concourse import mybir

B, N, C = 4, 2048, 1024
H, DH = 16, 64
HG = 8                # heads per core
HD = HG * DH          # 512 head-dims per core
SCALE = DH ** -0.5    # 0.125
KT = 9                # contraction k-tiles for V matmul (8 x + 1 bias/ones)
NCORES = 8

F32 = mybir.dt.float32

# matmul operand dtype knob: mybir.dt.bfloat16 or mybir.dt.float32r
DT = mybir.dt.bfloat16
NPDT = ml_dtypes.bfloat16 if DT == mybir.dt.bfloat16 else np.float32


def _replace_sem_range_clear(nc):
    """This walrus build rejects the EVENT_SEMAPHORE_RANGE_CLEAR InstISA that
    TileContext emits at kernel end. Replace it with per-semaphore negative
    sem-inc updates (attached to cheap Pool-engine carriers) that bring every
    kernel semaphore back to zero — equivalent effect, using only encodings
    this compiler accepts. Runs before _split_multi_waits."""
    f = nc.m.functions[0]
    blocks = list(f.blocks)
    snaps = [list(b.instructions) for b in blocks]
    totals = {}
    for insts in snaps:
        for i in insts:
            si = i.sync_info
            if si:
                for u in si.on_update:
                    if u.sync_type == "semaphore":
                        totals[u.id] = totals.get(u.id, 0) + u.update_value
    newlists = []
    for insts in snaps:
        newlist = []
        for i in insts:
            if type(i).__name__ == "InstISA" and "RANGE_CLEAR" in (i.op_name or ""):
                d = i.ant_dict
                for sem in range(d["range_first"], d["range_last"] + 1):
                    v = totals.get(sem, 0)
                    if v == 0:
                        continue
                    car = mybir.InstEventSemaphore(
                        name=nc.get_next_instruction_name()
                    )
                    car.engine = i.engine
                    car.sync_info = mybir.SyncInfo(
                        on_wait=[],
                        on_update=[
                            mybir.SyncUpdate(
                                sync_type="semaphore",
                                id=sem,
                                update_mode="sem-wr-imm",
                                update_value=0,
                                update_reg=None,
                            )
                        ],
                    )
                    newlist.append(car)
                continue  # drop the RANGE_CLEAR itself
            newlist.append(i)
        newlists.append(newlist)
    for b, nl in zip(blocks, newlists):
        b.instructions = nl


def _split_multi_waits(nc):
    """Legalize for walrus builds that allow only ONE sync wait per
    instruction: hoist extra waits onto cheap same-engine *real* carrier
    instructions inserted immediately before the offending instruction.
    A wait executed earlier in the same engine stream is strictly more
    conservative, so semantics are preserved.

    For matmuls, walrus encodes the matmul's syncs into its paired
    LDWEIGHTS struct, so the (LDW, MM) pair is treated as having capacity
    for ONE wait total; extras go onto scratch-LDWEIGHTS carriers placed
    before the pair (a stray weight load between complete pairs is
    harmless — every real matmul reloads its own weights)."""
    def make_carrier(engine):
        car = mybir.InstEventSemaphore(name=nc.get_next_instruction_name())
        car.engine = engine
        return car

    f = nc.m.functions[0]
    blocks = list(f.blocks)
    snapshots = [list(b.instructions) for b in blocks]
    newlists = []
    for insts in snapshots:
        newlist = []
        for i in insts:
            si = i.sync_info
            ty = type(i).__name__
            if si is not None and len(si.on_wait) > 1:
                waits = list(si.on_wait)
                is_mm = ty == "InstMatmult"
                # matmul syncs share the paired LDW's single wait slot, which
                # the LDW may already use — keep none on the matmul itself
                keep = 0 if is_mm else 1
                extras = waits[: len(waits) - keep]
                kept = waits[len(waits) - keep:]
                # insertion position: before the paired LDW for matmuls
                pos = len(newlist)
                if is_mm and pos > 0 and type(newlist[-1]).__name__ == "InstLdweights":
                    pos -= 1
                carriers = []
                for w in extras:
                    car = make_carrier(i.engine)
                    if car is None:
                        kept = waits  # cannot split; leave untouched
                        carriers = []
                        break
                    car.sync_info = mybir.SyncInfo(on_wait=[w], on_update=[])
                    carriers.append(car)
                if carriers or len(kept) < len(waits):
                    newlist[pos:pos] = carriers
                    i.sync_info = mybir.SyncInfo(
                        on_wait=kept, on_update=list(si.on_update)
                    )
            newlist.append(i)
        newlists.append(newlist)
    # assigning every block's list also wipes the stray auto-appended carriers
    for b, nl in zip(blocks, newlists):
        b.instructions = nl


def build_bass():
    nc = bass.Bass()

    xT = nc.declare_dram_parameter("xT", [KT * 128, N], DT, isOutput=False)
    wqk = nc.declare_dram_parameter("wqk", [C, 1024], DT, isOutput=False)
    wv = nc.declare_dram_parameter("wv", [KT * 128, HD], DT, isOutput=False)
    bqk = nc.declare_dram_parameter("bqk", [128, 8], F32, isOutput=False)
    wp = nc.declare_dram_parameter("wp", [HD, C], DT, isOutput=False)
    out = nc.declare_dram_parameter("out", [N, C], F32, isOutput=True)

    with tile.TileContext(nc) as tc, ExitStack() as ctx:
        res = ctx.enter_context(tc.tile_pool(name="res", bufs=1))
        ppool = ctx.enter_context(tc.tile_pool(name="ppool", bufs=4))
        spool = ctx.enter_context(tc.tile_pool(name="spool", bufs=2))
        opool = ctx.enter_context(tc.tile_pool(name="opool", bufs=2))
        ps_s = ctx.enter_context(tc.tile_pool(name="ps_s", bufs=2, space="PSUM"))
        ps_o = ctx.enter_context(tc.tile_pool(name="ps_o", bufs=3, space="PSUM"))
        ps_mm = ctx.enter_context(tc.tile_pool(name="ps_mm", bufs=1, space="PSUM"))
        dpool = ctx.enter_context(tc.tile_pool(name="dpool", bufs=4, space="DRAM"))

        # ---- resident SBUF tensors ----
        xT_sb = [res.tile([128, N], DT, name=f"xt{k}", tag=f"xt{k}") for k in range(KT)]
        wqk_sb = [res.tile([128, 1024], DT, name=f"wqk{k}", tag=f"wqk{k}") for k in range(8)]
        wv_sb = [res.tile([128, HD], DT, name=f"wv{k}", tag=f"wv{k}") for k in range(KT)]
        wp_sb = [res.tile([128, C], DT, name=f"wp{t}", tag=f"wp{t}") for t in range(4)]
        bqk_sb = res.tile([128, 8], F32, name="bqk_sb", tag="bqk_sb")
        qt_sb = [res.tile([128, N], DT, name=f"qt{t}", tag=f"qt{t}") for t in range(4)]
        kt_sb = [res.tile([128, N], DT, name=f"kt{t}", tag=f"kt{t}") for t in range(4)]
        vaug_sb = [res.tile([128, HG, DH + 1], DT, name=f"va{m}", tag=f"va{m}") for m in range(16)]
        onT_sb = [res.tile([128, N], DT, name=f"ot{t}", tag=f"ot{t}") for t in range(4)]

        # DMA issue order matches consumption order: bias, then the wqk
        # column slices for kt[0] (ct=4) and qt[0] (ct=0), then xT chunk 0
        # and the V weights (pre-phase), then the rest.
        nc.sync.dma_start(out=bqk_sb, in_=bqk[:, :])
        for k in range(8):
            nc.sync.dma_start(
                out=wqk_sb[k][:, 512:640], in_=wqk[k * 128:(k + 1) * 128, 512:640]
            )
        for k in range(8):
            nc.sync.dma_start(
                out=wqk_sb[k][:, 0:128], in_=wqk[k * 128:(k + 1) * 128, 0:128]
            )
        for k in range(KT):
            nc.sync.dma_start(
                out=xT_sb[k][:, 0:512], in_=xT[k * 128:(k + 1) * 128, 0:512]
            )
        for k in range(KT):
            nc.sync.dma_start(out=wv_sb[k], in_=wv[k * 128:(k + 1) * 128, :])
        for j in range(1, 4):
            for k in range(KT):
                nc.sync.dma_start(
                    out=xT_sb[k][:, j * 512:(j + 1) * 512],
                    in_=xT[k * 128:(k + 1) * 128, j * 512:(j + 1) * 512],
                )
        for k in range(8):
            nc.sync.dma_start(
                out=wqk_sb[k][:, 128:512], in_=wqk[k * 128:(k + 1) * 128, 128:512]
            )
            nc.sync.dma_start(
                out=wqk_sb[k][:, 640:1024], in_=wqk[k * 128:(k + 1) * 128, 640:1024]
            )
        for t in range(4):
            nc.sync.dma_start(out=wp_sb[t], in_=wp[t * 128:(t + 1) * 128, :])

        # ---- phase builders ----
        def qk_mms(ps, j, ct, k0, k1):
            for k in range(k0, k1):
                nc.tensor.matmul(
                    ps,
                    lhsT=wqk_sb[k][:, ct * 128:(ct + 1) * 128],
                    rhs=xT_sb[k][:, j * 512:(j + 1) * 512],
                    start=(k == 0),
                    stop=(k == 7),
                )

        def qk_copy(ps, j, ct):
            dst = qt_sb[ct] if ct < 4 else kt_sb[ct - 4]
            nc.vector.tensor_scalar_add(
                out=dst[:, j * 512:(j + 1) * 512],
                in0=ps,
                scalar1=bqk_sb[:, ct:ct + 1],
            )

        def qk_group_now(j, ct, pool):
            # pre-phase version: borrow the (otherwise idle) ps_s slots
            ps = pool.tile([128, 1024], F32, name=f"qkn{ct}_{j}", tag="ps")
            qk_mms(ps[:, 0:512], j, ct, 0, 8)
            qk_copy(ps[:, 0:512], j, ct)

        def gen_qk_group(j, ct):
            # filler version: small quanta on the 1-bank ps_mm pool
            ps = ps_mm.tile([128, 512], F32, name=f"qkg{ct}_{j}", tag="mm")
            for k0 in range(0, 8, 2):
                qk_mms(ps, j, ct, k0, k0 + 2)
                yield
            qk_copy(ps, j, ct)
            yield

        def v_tile_now(mt, pool):
            # V_aug [m, h, d|1] = x @ w_v (+ b_v via ones row)
            ps = pool.tile([128, 1024], F32, name=f"v_ps{mt}", tag="ps")
            for k in range(KT):
                nc.tensor.matmul(
                    ps[:, 0:512],
                    lhsT=xT_sb[k][:, mt * 128:(mt + 1) * 128],
                    rhs=wv_sb[k],
                    start=(k == 0),
                    stop=(k == KT - 1),
                )
            va = vaug_sb[mt]
            nc.vector.memset(va[:, :, DH:DH + 1], 1.0)
            nc.vector.tensor_copy(
                out=va[:, :, 0:DH],
                in_=ps[:, 0:512].rearrange("p (h d) -> p h d", h=HG),
            )

        ob_cur = {}

        def gen_proj_group(nt, cc):
            # one (n-tile, c-chunk) group of the proj partial
            if cc == 0:
                ob_cur[nt] = opool.tile([128, C], F32, name=f"ob{nt}", tag="ob")
            ob = ob_cur[nt]
            py = ps_mm.tile([128, 512], F32, name=f"y_ps{nt}_{cc}", tag="mm")
            for t in range(4):
                nc.tensor.matmul(
                    py,
                    lhsT=onT_sb[t][:, nt * 128:(nt + 1) * 128],
                    rhs=wp_sb[t][:, cc * 512:(cc + 1) * 512],
                    start=(t == 0),
                    stop=(t == 3),
                )
                if t == 1:
                    yield
            nc.vector.tensor_copy(out=ob[:, cc * 512:(cc + 1) * 512], in_=py)
            if cc == 1:
                nc.sync.dma_start(out=out[nt * 128:(nt + 1) * 128, :], in_=ob)
            yield

        def gen_delay(n):
            for _ in range(n):
                yield

        # ---- filler machinery: one quantum (~0.2-0.4us of PE) per step ----
        fillers = deque()
        cur_gen = [None]

        def emit_filler():
            while True:
                if cur_gen[0] is None:
                    if not fillers:
                        return
                    cur_gen[0] = fillers.popleft()
                try:
                    next(cur_gen[0])
                    return
                except StopIteration:
                    cur_gen[0] = None

        # ---- attention: pair-outer, query-chunk inner ----
        def attention_pair(u):
            for j in range(4):
                nsl = slice(j * 512, (j + 1) * 512)
                po_a = ps_o.tile([DH + 1, 512], F32, name=f"poa{u}_{j}", tag="po")
                po_b = ps_o.tile([DH + 1, 512], F32, name=f"pob{u}_{j}", tag="po")
                pts = {}
                for i in range(17):
                    if i < 16:
                        ps = ps_s.tile(
                            [128, 1024], F32, name=f"s_ps{u}_{j}_{i}", tag="ps"
                        )
                        # the two heads' S^T matmuls use disjoint PE row
                        # groups (k-dims 0-63 vs 64-127) -> run concurrently
                        nc.tensor.matmul(
                            ps[:, 0:512],
                            lhsT=kt_sb[u][0:64, i * 128:(i + 1) * 128],
                            rhs=qt_sb[u][0:64, nsl],
                            start=True,
                            stop=True,
                        )
                        nc.tensor.matmul(
                            ps[:, 512:1024],
                            lhsT=kt_sb[u][64:128, i * 128:(i + 1) * 128],
                            rhs=qt_sb[u][64:128, nsl],
                            start=True,
                            stop=True,
                        )
                        pt = ppool.tile(
                            [128, 1024], DT, name=f"pt{u}_{j}_{i}", tag="pt"
                        )
                        nc.scalar.activation(
                            out=pt, in_=ps, func=mybir.ActivationFunctionType.Exp
                        )
                        pts[i] = pt
                    if i >= 1:
                        mp = i - 1
                        pt = pts.pop(mp)
                        nc.tensor.matmul(
                            po_a,
                            lhsT=vaug_sb[mp][:, 2 * u, :],
                            rhs=pt[:, 0:512],
                            start=(mp == 0),
                            stop=(mp == 15),
                        )
                        nc.tensor.matmul(
                            po_b,
                            lhsT=vaug_sb[mp][:, 2 * u + 1, :],
                            rhs=pt[:, 512:1024],
                            start=(mp == 0),
                            stop=(mp == 15),
                        )
                    emit_filler()
                # normalization: row 64 holds the softmax denominators
                o_un = spool.tile([DH + 1, 1024], F32, name=f"ou{u}_{j}", tag="oun")
                nc.vector.tensor_copy(out=o_un[:, 0:512], in_=po_a)
                nc.vector.tensor_copy(out=o_un[:, 512:1024], in_=po_b)
                rrow = spool.tile([1, 1024], F32, name=f"rr{u}_{j}", tag="rrow")
                nc.vector.reciprocal(out=rrow, in_=o_un[DH:DH + 1, :])
                # broadcast 1/s across 64 partitions: bounce through DRAM and
                # re-read with a partition-stride-0 access pattern
                rdram = dpool.tile([1, 1024], F32, name=f"rd{u}_{j}", tag="rd")
                nc.sync.dma_start(out=rdram, in_=rrow)
                rbc = spool.tile([64, 1024], F32, name=f"rb{u}_{j}", tag="rbc")
                bc_ap = bass.AP(
                    tensor=rdram.tensor,
                    offset=rdram.offset,
                    ap=[[0, 64]] + [list(d) for d in rdram.ap[1:]],
                )
                nc.sync.dma_start(out=rbc, in_=bc_ap)
                nc.vector.tensor_tensor(
                    out=onT_sb[u][0:64, nsl],
                    in0=o_un[0:DH, 0:512],
                    in1=rbc[:, 0:512],
                    op=mybir.AluOpType.mult,
                )
                nc.vector.tensor_tensor(
                    out=onT_sb[u][64:128, nsl],
                    in0=o_un[0:DH, 512:1024],
                    in1=rbc[:, 512:1024],
                    op=mybir.AluOpType.mult,
                )
                if u == 3:
                    # proj for this query chunk's n-tiles is now unblocked;
                    # delay a few quanta so the norm chain (DVE+DMA bounce)
                    # lands before the first proj matmul reads onT
                    fillers.append(gen_delay(4))
                    for nt in range(j * 4, j * 4 + 4):
                        fillers.append(gen_proj_group(nt, 0))
                        fillers.append(gen_proj_group(nt, 1))

        # ---- schedule ----
        # pre-phase: kt[0] (all key chunks), qt[0] chunk 0, full V
        for j in range(4):
            qk_group_now(j, 4, ps_s)
        qk_group_now(0, 0, ps_s)
        for mt in range(16):
            v_tile_now(mt, ps_s)

        # filler queue: remaining Q^T chunks for pair 0, then K^T/Q^T for
        # later pairs (consumed during earlier pairs' ACT-bound attention)
        for j in range(1, 4):
            fillers.append(gen_qk_group(j, 0))
        for u in range(1, 4):
            for j in range(4):
                fillers.append(gen_qk_group(j, 4 + u))
            for j in range(4):
                fillers.append(gen_qk_group(j, u))

        for u in range(4):
            attention_pair(u)

        # drain any remaining filler work (tail proj groups)
        while fillers or cur_gen[0] is not None:
            if cur_gen[0] is None:
                cur_gen[0] = fillers.popleft()
            for _ in cur_gen[0]:
                pass
            cur_gen[0] = None

    _replace_sem_range_clear(nc)
    _split_multi_waits(nc)
    return nc


_NC_CACHE = None


def _get_nc():
    global _NC_CACHE
    if _NC_CACHE is None:
        _NC_CACHE = build_bass()
    return _NC_CACHE


def make_in_maps(x, w_qkv, b_qkv, w_proj):
    """Host-side sharding: returns the 8 per-core input dicts."""
    x = np.asarray(x, np.float32)
    w_qkv = np.asarray(w_qkv, np.float32)
    b_qkv = np.asarray(b_qkv, np.float32)
    w_proj = np.asarray(w_proj, np.float32)

    in_maps = []
    for core in range(NCORES):
        b, g = divmod(core, 2)
        cs = slice(512 * g, 512 * g + 512)

        wq = w_qkv[:, 0:1024][:, cs] * SCALE
        wk = w_qkv[:, 1024:2048][:, cs]
        wv_s = w_qkv[:, 2048:3072][:, cs]
        bq = b_qkv[0:1024][cs] * SCALE
        bk = b_qkv[1024:2048][cs]
        bv = b_qkv[2048:3072][cs]

        xT_aug = np.zeros((KT * 128, N), np.float32)
        xT_aug[:C] = x[b].T
        xT_aug[C] = 1.0

        wv_aug = np.zeros((KT * 128, HD), np.float32)
        wv_aug[:C] = wv_s
        wv_aug[C] = bv

        bqk_np = np.concatenate([bq, bk]).reshape(8, 128).T.copy()

        in_maps.append({
            "xT": xT_aug.astype(NPDT),
            "wqk": np.concatenate([wq, wk], axis=1).astype(NPDT),
            "wv": wv_aug.astype(NPDT),
            "bqk": np.ascontiguousarray(bqk_np, np.float32),
            "wp": w_proj[cs, :].astype(NPDT),
        })
    return in_maps


def assemble_output(results, b_proj):
    b_proj = np.asarray(b_proj, np.float32)
    outs = [np.asarray(r["out"], np.float32) for r in results]
    return np.stack([outs[2 * b] + outs[2 * b + 1] + b_proj for b in range(B)])


def run(x, w_qkv, b_qkv, w_proj, b_proj, **spmd_kwargs):
    from concourse.bass_utils import run_bass_kernel_spmd

    nc = _get_nc()
    in_maps = make_in_maps(x, w_qkv, b_qkv, w_proj)
    res = run_bass_kernel_spmd(nc, in_maps, list(range(NCORES)), **spmd_kwargs)
    return assemble_output(res.results, b_proj), res


def kernel(x, w_qkv, b_qkv, w_proj, b_proj):
    out, _ = run(x, w_qkv, b_qkv, w_proj, b_proj)
    return out


# revision 4
# speedup vs baseline: 1.0149x; 1.0149x over previous
"""Multi-head self-attention (B=4, N=2048, C=1024, H=16) on 8 Trainium2 cores.

Sharding: core = (batch b, head-group g) with b in 0..3, g in 0..1.
Each core computes, for its batch and its 8 heads:
    QKV projection -> per-head attention (S^T layout softmax) -> proj partial.
Host sums the two head-group partials per batch and adds b_proj.

v2 schedule: heads are processed in PAIRS (2u, 2u+1). The two S^T matmuls
of a pair use disjoint PE row-groups (k-dims at partitions 0-63 vs 64-127),
so the hardware runs them concurrently (~2x on the QK^T stage). Each
[128 keys, 1024] PSUM tile holds S^T for both heads (512 queries each) and
is consumed by ONE wide exp on ACT - the exp stream is the kernel's
critical path, so everything else (QKV projection groups, proj groups) is
chopped into small "filler quanta" interleaved between attention steps to
soak up the PE slack without stalling ACT.

Device-side layout choices (all transposes done on host, none on device):
  - x is shipped pre-transposed as xT [C, N] (+ a ones row for the V bias).
  - Q^T/K^T are produced as [c', n] tiles directly (lhsT = w_qk natural).
  - V is produced in natural [m, h*d] layout augmented with a ones column per
    head; the ones column makes the PV matmul emit the softmax row-sums.
  - Softmax runs on S^T tiles [m, n]: exp on the scalar engine, sums via the
    V ones-column, normalization via reciprocal + partition-broadcast + mult.
  - Projection consumes O^T [hd, n] tiles directly as lhsT.
"""

import os
import sys

if "/opt/trn_rl_repo" not in sys.path:
    sys.path.insert(0, "/opt/trn_rl_repo")

# the kernel executes through PJRT on the axon-tunneled NeuronCores; a
# cpu-pinned JAX_PLATFORMS (as some harnesses set for the reference) would
# hide the devices — fix it before anything imports jax
if "axon" not in os.environ.get("JAX_PLATFORMS", "axon"):
    os.environ["JAX_PLATFORMS"] = "axon"

from collections import deque
from contextlib import ExitStack

import ml_dtypes
import numpy as np

import concourse.bass as bass
import concourse.tile as tile
from concourse import mybir

B, N, C = 4, 2048, 1024
H, DH = 16, 64
HG = 8                # heads per core
HD = HG * DH          # 512 head-dims per core
SCALE = DH ** -0.5    # 0.125
KT = 9                # contraction k-tiles for V matmul (8 x + 1 bias/ones)
NCORES = 8

F32 = mybir.dt.float32

# matmul operand dtype knob: mybir.dt.bfloat16 or mybir.dt.float32r
DT = mybir.dt.bfloat16
NPDT = ml_dtypes.bfloat16 if DT == mybir.dt.bfloat16 else np.float32


def _replace_sem_range_clear(nc):
    """This walrus build rejects the EVENT_SEMAPHORE_RANGE_CLEAR InstISA that
    TileContext emits at kernel end. Replace it with per-semaphore negative
    sem-inc updates (attached to cheap Pool-engine carriers) that bring every
    kernel semaphore back to zero — equivalent effect, using only encodings
    this compiler accepts. Runs before _split_multi_waits."""
    f = nc.m.functions[0]
    blocks = list(f.blocks)
    snaps = [list(b.instructions) for b in blocks]
    totals = {}
    for insts in snaps:
        for i in insts:
            si = i.sync_info
            if si:
                for u in si.on_update:
                    if u.sync_type == "semaphore":
                        totals[u.id] = totals.get(u.id, 0) + u.update_value
    newlists = []
    for insts in snaps:
        newlist = []
        for i in insts:
            if type(i).__name__ == "InstISA" and "RANGE_CLEAR" in (i.op_name or ""):
                d = i.ant_dict
                for sem in range(d["range_first"], d["range_last"] + 1):
                    v = totals.get(sem, 0)
                    if v == 0:
                        continue
                    car = mybir.InstEventSemaphore(
                        name=nc.get_next_instruction_name()
                    )
                    car.engine = i.engine
                    car.sync_info = mybir.SyncInfo(
                        on_wait=[],
                        on_update=[
                            mybir.SyncUpdate(
                                sync_type="semaphore",
                                id=sem,
                                update_mode="sem-wr-imm",
                                update_value=0,
                                update_reg=None,
                            )
                        ],
                    )
                    newlist.append(car)
                continue  # drop the RANGE_CLEAR itself
            newlist.append(i)
        newlists.append(newlist)
    for b, nl in zip(blocks, newlists):
        b.instructions = nl


def _split_multi_waits(nc):
    """Legalize for walrus builds that allow only ONE sync wait per
    instruction: hoist extra waits onto cheap same-engine *real* carrier
    instructions inserted immediately before the offending instruction.
    A wait executed earlier in the same engine stream is strictly more
    conservative, so semantics are preserved.

    For matmuls, walrus encodes the matmul's syncs into its paired
    LDWEIGHTS struct, so the (LDW, MM) pair is treated as having capacity
    for ONE wait total; extras go onto scratch-LDWEIGHTS carriers placed
    before the pair (a stray weight load between complete pairs is
    harmless — every real matmul reloads its own weights)."""
    def make_carrier(engine):
        car = mybir.InstEventSemaphore(name=nc.get_next_instruction_name())
        car.engine = engine
        return car

    f = nc.m.functions[0]
    blocks = list(f.blocks)
    snapshots = [list(b.instructions) for b in blocks]
    newlists = []
    for insts in snapshots:
        newlist = []
        for i in insts:
            si = i.sync_info
            ty = type(i).__name__
            if si is not None and len(si.on_wait) > 1:
                waits = list(si.on_wait)
                is_mm = ty == "InstMatmult"
                # matmul syncs share the paired LDW's single wait slot, which
                # the LDW may already use — keep none on the matmul itself
                keep = 0 if is_mm else 1
                extras = waits[: len(waits) - keep]
                kept = waits[len(waits) - keep:]
                # insertion position: before the paired LDW for matmuls
                pos = len(newlist)
                if is_mm and pos > 0 and type(newlist[-1]).__name__ == "InstLdweights":
                    pos -= 1
                carriers = []
                for w in extras:
                    car = make_carrier(i.engine)
                    if car is None:
                        kept = waits  # cannot split; leave untouched
                        carriers = []
                        break
                    car.sync_info = mybir.SyncInfo(on_wait=[w], on_update=[])
                    carriers.append(car)
                if carriers or len(kept) < len(waits):
                    newlist[pos:pos] = carriers
                    i.sync_info = mybir.SyncInfo(
                        on_wait=kept, on_update=list(si.on_update)
                    )
            newlist.append(i)
        newlists.append(newlist)
    # assigning every block's list also wipes the stray auto-appended carriers
    for b, nl in zip(blocks, newlists):
        b.instructions = nl


def build_bass():
    nc = bass.Bass()

    xT = nc.declare_dram_parameter("xT", [KT * 128, N], DT, isOutput=False)
    wqk = nc.declare_dram_parameter("wqk", [C, 1024], DT, isOutput=False)
    wv = nc.declare_dram_parameter("wv", [KT * 128, HD], DT, isOutput=False)
    bqk = nc.declare_dram_parameter("bqk", [128, 8], F32, isOutput=False)
    wp = nc.declare_dram_parameter("wp", [HD, C], DT, isOutput=False)
    out = nc.declare_dram_parameter("out", [N, C], F32, isOutput=True)

    with tile.TileContext(nc) as tc, ExitStack() as ctx:
        res = ctx.enter_context(tc.tile_pool(name="res", bufs=1))
        ppool = ctx.enter_context(tc.tile_pool(name="ppool", bufs=4))
        spool = ctx.enter_context(tc.tile_pool(name="spool", bufs=2))
        opool = ctx.enter_context(tc.tile_pool(name="opool", bufs=2))
        ps_s = ctx.enter_context(tc.tile_pool(name="ps_s", bufs=2, space="PSUM"))
        ps_o = ctx.enter_context(tc.tile_pool(name="ps_o", bufs=3, space="PSUM"))
        ps_mm = ctx.enter_context(tc.tile_pool(name="ps_mm", bufs=1, space="PSUM"))
        dpool = ctx.enter_context(tc.tile_pool(name="dpool", bufs=4, space="DRAM"))

        # ---- resident SBUF tensors ----
        xT_sb = [res.tile([128, N], DT, name=f"xt{k}", tag=f"xt{k}") for k in range(KT)]
        wqk_sb = [res.tile([128, 1024], DT, name=f"wqk{k}", tag=f"wqk{k}") for k in range(8)]
        wv_sb = [res.tile([128, HD], DT, name=f"wv{k}", tag=f"wv{k}") for k in range(KT)]
        wp_sb = [res.tile([128, C], DT, name=f"wp{t}", tag=f"wp{t}") for t in range(4)]
        bqk_sb = res.tile([128, 8], F32, name="bqk_sb", tag="bqk_sb")
        qt_sb = [res.tile([128, N], DT, name=f"qt{t}", tag=f"qt{t}") for t in range(4)]
        kt_sb = [res.tile([128, N], DT, name=f"kt{t}", tag=f"kt{t}") for t in range(4)]
        vaug_sb = [res.tile([128, HG, DH + 1], DT, name=f"va{m}", tag=f"va{m}") for m in range(16)]
        onT_sb = [res.tile([128, N], DT, name=f"ot{t}", tag=f"ot{t}") for t in range(4)]

        # DMA issue order matches consumption order: bias, then the wqk
        # column slices for kt[0] (ct=4) and qt[0] (ct=0), then xT chunk 0
        # and the V weights (pre-phase), then the rest.
        nc.sync.dma_start(out=bqk_sb, in_=bqk[:, :])
        for k in range(8):
            nc.sync.dma_start(
                out=wqk_sb[k][:, 512:640], in_=wqk[k * 128:(k + 1) * 128, 512:640]
            )
        for k in range(8):
            nc.sync.dma_start(
                out=wqk_sb[k][:, 0:128], in_=wqk[k * 128:(k + 1) * 128, 0:128]
            )
        for k in range(KT):
            nc.sync.dma_start(
                out=xT_sb[k][:, 0:512], in_=xT[k * 128:(k + 1) * 128, 0:512]
            )
        for k in range(KT):
            nc.sync.dma_start(out=wv_sb[k], in_=wv[k * 128:(k + 1) * 128, :])
        for j in range(1, 4):
            for k in range(KT):
                nc.sync.dma_start(
                    out=xT_sb[k][:, j * 512:(j + 1) * 512],
                    in_=xT[k * 128:(k + 1) * 128, j * 512:(j + 1) * 512],
                )
        for k in range(8):
            nc.sync.dma_start(
                out=wqk_sb[k][:, 128:512], in_=wqk[k * 128:(k + 1) * 128, 128:512]
            )
            nc.sync.dma_start(
                out=wqk_sb[k][:, 640:1024], in_=wqk[k * 128:(k + 1) * 128, 640:1024]
            )
        for t in range(4):
            nc.sync.dma_start(out=wp_sb[t], in_=wp[t * 128:(t + 1) * 128, :])

        # ---- phase builders ----
        def qk_mms(ps, j, ct, k0, k1):
            for k in range(k0, k1):
                nc.tensor.matmul(
                    ps,
                    lhsT=wqk_sb[k][:, ct * 128:(ct + 1) * 128],
                    rhs=xT_sb[k][:, j * 512:(j + 1) * 512],
                    start=(k == 0),
                    stop=(k == 7),
                )

        def qk_copy(ps, j, ct):
            dst = qt_sb[ct] if ct < 4 else kt_sb[ct - 4]
            nc.vector.tensor_scalar_add(
                out=dst[:, j * 512:(j + 1) * 512],
                in0=ps,
                scalar1=bqk_sb[:, ct:ct + 1],
            )

        def qk_group_now(j, ct, pool):
            # pre-phase version: borrow the (otherwise idle) ps_s slots
            ps = pool.tile([128, 1024], F32, name=f"qkn{ct}_{j}", tag="ps")
            qk_mms(ps[:, 0:512], j, ct, 0, 8)
            qk_copy(ps[:, 0:512], j, ct)

        def gen_qk_group(j, ct):
            # filler version: small quanta on the 1-bank ps_mm pool
            ps = ps_mm.tile([128, 512], F32, name=f"qkg{ct}_{j}", tag="mm")
            for k0 in range(0, 8, 2):
                qk_mms(ps, j, ct, k0, k0 + 2)
                yield
            qk_copy(ps, j, ct)
            yield

        def v_tile_now(mt, pool):
            # V_aug [m, h, d|1] = x @ w_v (+ b_v via ones row)
            ps = pool.tile([128, 1024], F32, name=f"v_ps{mt}", tag="ps")
            for k in range(KT):
                nc.tensor.matmul(
                    ps[:, 0:512],
                    lhsT=xT_sb[k][:, mt * 128:(mt + 1) * 128],
                    rhs=wv_sb[k],
                    start=(k == 0),
                    stop=(k == KT - 1),
                )
            va = vaug_sb[mt]
            nc.vector.memset(va[:, :, DH:DH + 1], 1.0)
            nc.vector.tensor_copy(
                out=va[:, :, 0:DH],
                in_=ps[:, 0:512].rearrange("p (h d) -> p h d", h=HG),
            )

        ob_cur = {}

        def gen_proj_group(nt, cc):
            # one (n-tile, c-chunk) group of the proj partial
            if cc == 0:
                ob_cur[nt] = opool.tile([128, C], F32, name=f"ob{nt}", tag="ob")
            ob = ob_cur[nt]
            py = ps_mm.tile([128, 512], F32, name=f"y_ps{nt}_{cc}", tag="mm")
            for t in range(4):
                nc.tensor.matmul(
                    py,
                    lhsT=onT_sb[t][:, nt * 128:(nt + 1) * 128],
                    rhs=wp_sb[t][:, cc * 512:(cc + 1) * 512],
                    start=(t == 0),
                    stop=(t == 3),
                )
                if t == 1:
                    yield
            nc.vector.tensor_copy(out=ob[:, cc * 512:(cc + 1) * 512], in_=py)
            if cc == 1:
                nc.sync.dma_start(out=out[nt * 128:(nt + 1) * 128, :], in_=ob)
            yield

        def gen_delay(n):
            for _ in range(n):
                yield

        # ---- filler machinery: one quantum (~0.2-0.4us of PE) per step ----
        fillers = deque()
        cur_gen = [None]

        def emit_filler():
            while True:
                if cur_gen[0] is None:
                    if not fillers:
                        return
                    cur_gen[0] = fillers.popleft()
                try:
                    next(cur_gen[0])
                    return
                except StopIteration:
                    cur_gen[0] = None

        # ---- attention: one global software pipeline over all chunks ----
        # chunk c = (pair u, query chunk j), pair-outer; 16 key-steps per
        # chunk; the S/exp stream runs D steps ahead of the PV stream so
        # chunk boundaries never bubble the ACT exp stream.
        CHUNKS = [(u, j) for u in range(4) for j in range(4)]
        NSTEP = len(CHUNKS) * 16
        pts = {}
        po_cur = {}

        def s_exp(s):
            c, i = divmod(s, 16)
            u, j = CHUNKS[c]
            nsl = slice(j * 512, (j + 1) * 512)
            ps = ps_s.tile([128, 1024], F32, name=f"s_ps{c}_{i}", tag="ps")
            # the two heads' S^T matmuls use disjoint PE row groups
            # (k-dims at partitions 0-63 vs 64-127) -> run concurrently
            nc.tensor.matmul(
                ps[:, 0:512],
                lhsT=kt_sb[u][0:64, i * 128:(i + 1) * 128],
                rhs=qt_sb[u][0:64, nsl],
                start=True,
                stop=True,
            )
            nc.tensor.matmul(
                ps[:, 512:1024],
                lhsT=kt_sb[u][64:128, i * 128:(i + 1) * 128],
                rhs=qt_sb[u][64:128, nsl],
                start=True,
                stop=True,
            )
            pt = ppool.tile([128, 1024], DT, name=f"pt{c}_{i}", tag="pt")
            nc.scalar.activation(
                out=pt, in_=ps, func=mybir.ActivationFunctionType.Exp
            )
            pts[s] = pt

        def norm_chunk(c):
            u, j = CHUNKS[c]
            nsl = slice(j * 512, (j + 1) * 512)
            po_a, po_b = po_cur.pop(c)
            # normalization: row 64 holds the softmax denominators
            o_un = spool.tile([DH + 1, 1024], F32, name=f"ou{c}", tag="oun")
            nc.vector.tensor_copy(out=o_un[:, 0:512], in_=po_a)
            nc.vector.tensor_copy(out=o_un[:, 512:1024], in_=po_b)
            rrow = spool.tile([1, 1024], F32, name=f"rr{c}", tag="rrow")
            nc.vector.reciprocal(out=rrow, in_=o_un[DH:DH + 1, :])
            # broadcast 1/s across 64 partitions: bounce through DRAM and
            # re-read with a partition-stride-0 access pattern
            rdram = dpool.tile([1, 1024], F32, name=f"rd{c}", tag="rd")
            nc.sync.dma_start(out=rdram, in_=rrow)
            rbc = spool.tile([64, 1024], F32, name=f"rb{c}", tag="rbc")
            bc_ap = bass.AP(
                tensor=rdram.tensor,
                offset=rdram.offset,
                ap=[[0, 64]] + [list(d) for d in rdram.ap[1:]],
            )
            nc.sync.dma_start(out=rbc, in_=bc_ap)
            nc.vector.tensor_tensor(
                out=onT_sb[u][0:64, nsl],
                in0=o_un[0:DH, 0:512],
                in1=rbc[:, 0:512],
                op=mybir.AluOpType.mult,
            )
            nc.vector.tensor_tensor(
                out=onT_sb[u][64:128, nsl],
                in0=o_un[0:DH, 512:1024],
                in1=rbc[:, 512:1024],
                op=mybir.AluOpType.mult,
            )
            if u == 3:
                # proj for this query chunk's n-tiles is now unblocked;
                # delay a few quanta so the norm chain (DVE+DMA bounce)
                # lands before the first proj matmul reads onT
                fillers.append(gen_delay(4))
                for nt in range(j * 4, j * 4 + 4):
                    fillers.append(gen_proj_group(nt, 0))
                    fillers.append(gen_proj_group(nt, 1))

        def pv(g):
            c, i = divmod(g, 16)
            u, j = CHUNKS[c]
            if i == 0:
                po_cur[c] = (
                    ps_o.tile([DH + 1, 512], F32, name=f"poa{c}", tag="po"),
                    ps_o.tile([DH + 1, 512], F32, name=f"pob{c}", tag="po"),
                )
            po_a, po_b = po_cur[c]
            pt = pts.pop(g)
            nc.tensor.matmul(
                po_a,
                lhsT=vaug_sb[i][:, 2 * u, :],
                rhs=pt[:, 0:512],
                start=(i == 0),
                stop=(i == 15),
            )
            nc.tensor.matmul(
                po_b,
                lhsT=vaug_sb[i][:, 2 * u + 1, :],
                rhs=pt[:, 512:1024],
                start=(i == 0),
                stop=(i == 15),
            )
            if i == 15:
                norm_chunk(c)

        # ---- schedule ----
        # pre-phase: kt[0] (all key chunks), qt[0] chunk 0, full V
        for j in range(4):
            qk_group_now(j, 4, ps_s)
        qk_group_now(0, 0, ps_s)
        for mt in range(16):
            v_tile_now(mt, ps_s)

        # filler queue: remaining Q^T chunks for pair 0, then K^T/Q^T for
        # later pairs (consumed during earlier pairs' ACT-bound attention)
        for j in range(1, 4):
            fillers.append(gen_qk_group(j, 0))
        for u in range(1, 4):
            for j in range(4):
                fillers.append(gen_qk_group(j, 4 + u))
            for j in range(4):
                fillers.append(gen_qk_group(j, u))

        # pipeline prologue: S/exp run D=2 steps ahead of PV
        s_exp(0)
        s_exp(1)
        for g in range(NSTEP):
            emit_filler()
            pv(g)
            if g + 2 < NSTEP:
                s_exp(g + 2)

        # drain any remaining filler work (tail proj groups)
        while fillers or cur_gen[0] is not None:
            if cur_gen[0] is None:
                cur_gen[0] = fillers.popleft()
            for _ in cur_gen[0]:
                pass
            cur_gen[0] = None

    _replace_sem_range_clear(nc)
    _split_multi_waits(nc)
    return nc


_NC_CACHE = None


def _get_nc():
    global _NC_CACHE
    if _NC_CACHE is None:
        _NC_CACHE = build_bass()
    return _NC_CACHE


def make_in_maps(x, w_qkv, b_qkv, w_proj):
    """Host-side sharding: returns the 8 per-core input dicts."""
    x = np.asarray(x, np.float32)
    w_qkv = np.asarray(w_qkv, np.float32)
    b_qkv = np.asarray(b_qkv, np.float32)
    w_proj = np.asarray(w_proj, np.float32)

    in_maps = []
    for core in range(NCORES):
        b, g = divmod(core, 2)
        cs = slice(512 * g, 512 * g + 512)

        wq = w_qkv[:, 0:1024][:, cs] * SCALE
        wk = w_qkv[:, 1024:2048][:, cs]
        wv_s = w_qkv[:, 2048:3072][:, cs]
        bq = b_qkv[0:1024][cs] * SCALE
        bk = b_qkv[1024:2048][cs]
        bv = b_qkv[2048:3072][cs]

        xT_aug = np.zeros((KT * 128, N), np.float32)
        xT_aug[:C] = x[b].T
        xT_aug[C] = 1.0

        wv_aug = np.zeros((KT * 128, HD), np.float32)
        wv_aug[:C] = wv_s
        wv_aug[C] = bv

        bqk_np = np.concatenate([bq, bk]).reshape(8, 128).T.copy()

        in_maps.append({
            "xT": xT_aug.astype(NPDT),
            "wqk": np.concatenate([wq, wk], axis=1).astype(NPDT),
            "wv": wv_aug.astype(NPDT),
            "bqk": np.ascontiguousarray(bqk_np, np.float32),
            "wp": w_proj[cs, :].astype(NPDT),
        })
    return in_maps


def assemble_output(results, b_proj):
    b_proj = np.asarray(b_proj, np.float32)
    outs = [np.asarray(r["out"], np.float32) for r in results]
    return np.stack([outs[2 * b] + outs[2 * b + 1] + b_proj for b in range(B)])


def run(x, w_qkv, b_qkv, w_proj, b_proj, **spmd_kwargs):
    from concourse.bass_utils import run_bass_kernel_spmd

    nc = _get_nc()
    in_maps = make_in_maps(x, w_qkv, b_qkv, w_proj)
    res = run_bass_kernel_spmd(nc, in_maps, list(range(NCORES)), **spmd_kwargs)
    return assemble_output(res.results, b_proj), res


def kernel(x, w_qkv, b_qkv, w_proj, b_proj):
    out, _ = run(x, w_qkv, b_qkv, w_proj, b_proj)
    return out


# revision 9
# speedup vs baseline: 1.1687x; 1.1516x over previous
"""Multi-head self-attention (B=4, N=2048, C=1024, H=16) on 8 Trainium2 cores.

Sharding: core = (batch b, head-group g) with b in 0..3, g in 0..1.
Each core computes, for its batch and its 8 heads:
    QKV projection -> per-head attention (S^T layout softmax) -> proj partial.
Host sums the two head-group partials per batch and adds b_proj.

v2 schedule: heads are processed in PAIRS (2u, 2u+1). The two S^T matmuls
of a pair use disjoint PE row-groups (k-dims at partitions 0-63 vs 64-127),
so the hardware runs them concurrently (~2x on the QK^T stage). Each
[128 keys, 1024] PSUM tile holds S^T for both heads (512 queries each) and
is consumed by ONE wide exp on ACT - the exp stream is the kernel's
critical path, so everything else (QKV projection groups, proj groups) is
chopped into small "filler quanta" interleaved between attention steps to
soak up the PE slack without stalling ACT.

Device-side layout choices (all transposes done on host, none on device):
  - x is shipped pre-transposed as xT [C, N] (+ a ones row for the V bias).
  - Q^T/K^T are produced as [c', n] tiles directly (lhsT = w_qk natural).
  - V is produced in natural [m, h*d] layout augmented with a ones column per
    head; the ones column makes the PV matmul emit the softmax row-sums.
  - Softmax runs on S^T tiles [m, n]: exp on the scalar engine, sums via the
    V ones-column, normalization via reciprocal + partition-broadcast + mult.
  - Projection consumes O^T [hd, n] tiles directly as lhsT.
"""

import os
import sys

if "/opt/trn_rl_repo" not in sys.path:
    sys.path.insert(0, "/opt/trn_rl_repo")

# the kernel executes through PJRT on the axon-tunneled NeuronCores; a
# cpu-pinned JAX_PLATFORMS (as some harnesses set for the reference) would
# hide the devices — fix it before anything imports jax
if "axon" not in os.environ.get("JAX_PLATFORMS", "axon"):
    os.environ["JAX_PLATFORMS"] = "axon"

from collections import deque
from contextlib import ExitStack

import ml_dtypes
import numpy as np

import concourse.bass as bass
import concourse.tile as tile
from concourse import mybir

B, N, C = 4, 2048, 1024
H, DH = 16, 64
HG = 8                # heads per core
HD = HG * DH          # 512 head-dims per core
SCALE = DH ** -0.5    # 0.125
KT = 9                # contraction k-tiles for V matmul (8 x + 1 bias/ones)
NCORES = 8

F32 = mybir.dt.float32

# matmul operand dtype knob: mybir.dt.bfloat16 or mybir.dt.float32r
DT = mybir.dt.bfloat16
NPDT = ml_dtypes.bfloat16 if DT == mybir.dt.bfloat16 else np.float32


def _replace_sem_range_clear(nc):
    """This walrus build rejects the EVENT_SEMAPHORE_RANGE_CLEAR InstISA that
    TileContext emits at kernel end. Replace it with per-semaphore negative
    sem-inc updates (attached to cheap Pool-engine carriers) that bring every
    kernel semaphore back to zero — equivalent effect, using only encodings
    this compiler accepts. Runs before _split_multi_waits."""
    f = nc.m.functions[0]
    blocks = list(f.blocks)
    snaps = [list(b.instructions) for b in blocks]
    totals = {}
    for insts in snaps:
        for i in insts:
            si = i.sync_info
            if si:
                for u in si.on_update:
                    if u.sync_type == "semaphore":
                        totals[u.id] = totals.get(u.id, 0) + u.update_value
    newlists = []
    for insts in snaps:
        newlist = []
        for i in insts:
            if type(i).__name__ == "InstISA" and "RANGE_CLEAR" in (i.op_name or ""):
                d = i.ant_dict
                for sem in range(d["range_first"], d["range_last"] + 1):
                    v = totals.get(sem, 0)
                    if v == 0:
                        continue
                    car = mybir.InstEventSemaphore(
                        name=nc.get_next_instruction_name()
                    )
                    car.engine = i.engine
                    car.sync_info = mybir.SyncInfo(
                        on_wait=[],
                        on_update=[
                            mybir.SyncUpdate(
                                sync_type="semaphore",
                                id=sem,
                                update_mode="sem-wr-imm",
                                update_value=0,
                                update_reg=None,
                            )
                        ],
                    )
                    newlist.append(car)
                continue  # drop the RANGE_CLEAR itself
            newlist.append(i)
        newlists.append(newlist)
    for b, nl in zip(blocks, newlists):
        b.instructions = nl


def _split_multi_waits(nc):
    """Legalize for walrus builds that allow only ONE sync wait per
    instruction: hoist extra waits onto cheap same-engine *real* carrier
    instructions inserted immediately before the offending instruction.
    A wait executed earlier in the same engine stream is strictly more
    conservative, so semantics are preserved.

    For matmuls, walrus encodes the matmul's syncs into its paired
    LDWEIGHTS struct, so the (LDW, MM) pair is treated as having capacity
    for ONE wait total; extras go onto scratch-LDWEIGHTS carriers placed
    before the pair (a stray weight load between complete pairs is
    harmless — every real matmul reloads its own weights)."""
    def make_carrier(engine):
        car = mybir.InstEventSemaphore(name=nc.get_next_instruction_name())
        car.engine = engine
        return car

    f = nc.m.functions[0]
    blocks = list(f.blocks)
    snapshots = [list(b.instructions) for b in blocks]
    newlists = []
    for insts in snapshots:
        newlist = []
        for i in insts:
            si = i.sync_info
            ty = type(i).__name__
            if si is not None and len(si.on_wait) > 1:
                waits = list(si.on_wait)
                is_mm = ty == "InstMatmult"
                # matmul syncs share the paired LDW's single wait slot, which
                # the LDW may already use — keep none on the matmul itself
                keep = 0 if is_mm else 1
                extras = waits[: len(waits) - keep]
                kept = waits[len(waits) - keep:]
                # insertion position: before the paired LDW for matmuls
                pos = len(newlist)
                if is_mm and pos > 0 and type(newlist[-1]).__name__ == "InstLdweights":
                    pos -= 1
                carriers = []
                for w in extras:
                    car = make_carrier(i.engine)
                    if car is None:
                        kept = waits  # cannot split; leave untouched
                        carriers = []
                        break
                    car.sync_info = mybir.SyncInfo(on_wait=[w], on_update=[])
                    carriers.append(car)
                if carriers or len(kept) < len(waits):
                    newlist[pos:pos] = carriers
                    i.sync_info = mybir.SyncInfo(
                        on_wait=kept, on_update=list(si.on_update)
                    )
            newlist.append(i)
        newlists.append(newlist)
    # assigning every block's list also wipes the stray auto-appended carriers
    for b, nl in zip(blocks, newlists):
        b.instructions = nl


def build_bass():
    nc = bass.Bass()

    xT = nc.declare_dram_parameter("xT", [KT * 128, N], DT, isOutput=False)
    wqk = nc.declare_dram_parameter("wqk", [C, 1024], DT, isOutput=False)
    wv = nc.declare_dram_parameter("wv", [KT * 128, HD], DT, isOutput=False)
    bqk = nc.declare_dram_parameter("bqk", [128, 8], F32, isOutput=False)
    wp = nc.declare_dram_parameter("wp", [HD, C], DT, isOutput=False)
    out = nc.declare_dram_parameter("out", [N, C], F32, isOutput=True)

    with tile.TileContext(nc) as tc, ExitStack() as ctx:
        res = ctx.enter_context(tc.tile_pool(name="res", bufs=1))
        ppool = ctx.enter_context(tc.tile_pool(name="ppool", bufs=4))
        spool = ctx.enter_context(tc.tile_pool(name="spool", bufs=2))
        opool = ctx.enter_context(tc.tile_pool(name="opool", bufs=2))
        ps_s = ctx.enter_context(tc.tile_pool(name="ps_s", bufs=2, space="PSUM"))
        ps_o = ctx.enter_context(tc.tile_pool(name="ps_o", bufs=3, space="PSUM"))
        ps_mm = ctx.enter_context(tc.tile_pool(name="ps_mm", bufs=1, space="PSUM"))
        dpool = ctx.enter_context(tc.tile_pool(name="dpool", bufs=4, space="DRAM"))

        # ---- resident SBUF tensors ----
        xT_sb = [res.tile([128, N], DT, name=f"xt{k}", tag=f"xt{k}") for k in range(KT)]
        wqk_sb = [res.tile([128, 1024], DT, name=f"wqk{k}", tag=f"wqk{k}") for k in range(8)]
        wv_sb = [res.tile([128, HD], DT, name=f"wv{k}", tag=f"wv{k}") for k in range(KT)]
        wp_sb = [res.tile([128, C], DT, name=f"wp{t}", tag=f"wp{t}") for t in range(4)]
        bqk_sb = res.tile([128, 8], F32, name="bqk_sb", tag="bqk_sb")
        qt_sb = [res.tile([128, N], DT, name=f"qt{t}", tag=f"qt{t}") for t in range(4)]
        kt_sb = [res.tile([128, N], DT, name=f"kt{t}", tag=f"kt{t}") for t in range(4)]
        vaug_sb = [res.tile([128, HG, DH + 1], DT, name=f"va{m}", tag=f"va{m}") for m in range(16)]
        onT_sb = [res.tile([128, N], DT, name=f"ot{t}", tag=f"ot{t}") for t in range(4)]

        # DMA issue order matches consumption order: bias, then per-k pairs
        # of (wqk kt[0]-slice, xT chunk-0 tile) so the first QK group's
        # matmuls start as soon as each k-tile lands, then qt[0] slices and
        # the V weights (pre-phase), then the rest.
        nc.sync.dma_start(out=bqk_sb, in_=bqk[:, :])
        for k in range(8):
            nc.sync.dma_start(
                out=wqk_sb[k][:, 512:640], in_=wqk[k * 128:(k + 1) * 128, 512:640]
            )
            nc.sync.dma_start(
                out=xT_sb[k][:, 0:512], in_=xT[k * 128:(k + 1) * 128, 0:512]
            )
        for k in range(8):
            nc.sync.dma_start(
                out=wqk_sb[k][:, 0:128], in_=wqk[k * 128:(k + 1) * 128, 0:128]
            )
        nc.sync.dma_start(out=xT_sb[8][:, 0:512], in_=xT[8 * 128:9 * 128, 0:512])
        for k in range(KT):
            nc.sync.dma_start(out=wv_sb[k], in_=wv[k * 128:(k + 1) * 128, :])
        for j in range(1, 4):
            for k in range(KT):
                nc.sync.dma_start(
                    out=xT_sb[k][:, j * 512:(j + 1) * 512],
                    in_=xT[k * 128:(k + 1) * 128, j * 512:(j + 1) * 512],
                )
        for k in range(8):
            nc.sync.dma_start(
                out=wqk_sb[k][:, 128:512], in_=wqk[k * 128:(k + 1) * 128, 128:512]
            )
            nc.sync.dma_start(
                out=wqk_sb[k][:, 640:1024], in_=wqk[k * 128:(k + 1) * 128, 640:1024]
            )
        for t in range(4):
            nc.sync.dma_start(out=wp_sb[t], in_=wp[t * 128:(t + 1) * 128, :])

        # ---- phase builders ----
        def qk_mms(ps, j, ct, k0, k1):
            for k in range(k0, k1):
                nc.tensor.matmul(
                    ps,
                    lhsT=wqk_sb[k][:, ct * 128:(ct + 1) * 128],
                    rhs=xT_sb[k][:, j * 512:(j + 1) * 512],
                    start=(k == 0),
                    stop=(k == 7),
                )

        def qk_copy(ps, j, ct):
            dst = qt_sb[ct] if ct < 4 else kt_sb[ct - 4]
            nc.vector.tensor_scalar_add(
                out=dst[:, j * 512:(j + 1) * 512],
                in0=ps,
                scalar1=bqk_sb[:, ct:ct + 1],
            )

        def qk_group_now(j, ct, pool):
            # pre-phase version: borrow the (otherwise idle) ps_s slots
            ps = pool.tile([128, 1024], F32, name=f"qkn{ct}_{j}", tag="ps")
            qk_mms(ps[:, 0:512], j, ct, 0, 8)
            qk_copy(ps[:, 0:512], j, ct)

        def gen_qk_group(j, ct):
            # filler version: small quanta on the 1-bank ps_mm pool
            ps = ps_mm.tile([128, 512], F32, name=f"qkg{ct}_{j}", tag="mm")
            for k0 in range(0, 8, 2):
                qk_mms(ps, j, ct, k0, k0 + 2)
                yield
            qk_copy(ps, j, ct)
            yield

        def v_tile_now(mt, pool):
            # V_aug [m, h, d|1] = x @ w_v (+ b_v via ones row)
            ps = pool.tile([128, 1024], F32, name=f"v_ps{mt}", tag="ps")
            for k in range(KT):
                nc.tensor.matmul(
                    ps[:, 0:512],
                    lhsT=xT_sb[k][:, mt * 128:(mt + 1) * 128],
                    rhs=wv_sb[k],
                    start=(k == 0),
                    stop=(k == KT - 1),
                )
            va = vaug_sb[mt]
            nc.vector.memset(va[:, :, DH:DH + 1], 1.0)
            nc.vector.tensor_copy(
                out=va[:, :, 0:DH],
                in_=ps[:, 0:512].rearrange("p (h d) -> p h d", h=HG),
            )

        def gen_proj_group(nt, cc):
            # one (n-tile, c-chunk) group of the proj partial; each half is
            # DMA'd out as soon as its copy lands so the output trickles out
            # instead of piling into the kernel tail
            ob = opool.tile([128, 512], F32, name=f"ob{nt}_{cc}", tag="ob")
            py = ps_mm.tile([128, 512], F32, name=f"y_ps{nt}_{cc}", tag="mm")
            for t in range(4):
                nc.tensor.matmul(
                    py,
                    lhsT=onT_sb[t][:, nt * 128:(nt + 1) * 128],
                    rhs=wp_sb[t][:, cc * 512:(cc + 1) * 512],
                    start=(t == 0),
                    stop=(t == 3),
                )
                if t == 1:
                    yield
            nc.vector.tensor_copy(out=ob, in_=py)
            nc.sync.dma_start(
                out=out[nt * 128:(nt + 1) * 128, cc * 512:(cc + 1) * 512], in_=ob
            )
            yield

        def gen_delay(n):
            for _ in range(n):
                yield

        # ---- filler machinery: one quantum (~0.2-0.4us of PE) per step ----
        fillers = deque()
        cur_gen = [None]

        def emit_filler():
            while True:
                if cur_gen[0] is None:
                    if not fillers:
                        return
                    cur_gen[0] = fillers.popleft()
                try:
                    next(cur_gen[0])
                    return
                except StopIteration:
                    cur_gen[0] = None

        # ---- attention: one global software pipeline over all chunks ----
        # chunk c = (pair u, query chunk j), pair-outer; 16 key-steps per
        # chunk; the S/exp stream runs D steps ahead of the PV stream so
        # chunk boundaries never bubble the ACT exp stream.
        CHUNKS = [(u, j) for u in range(4) for j in range(4)]
        NSTEP = len(CHUNKS) * 16
        pts = {}
        po_cur = {}

        def s_exp(s):
            c, i = divmod(s, 16)
            u, j = CHUNKS[c]
            nsl = slice(j * 512, (j + 1) * 512)
            ps = ps_s.tile([128, 1024], F32, name=f"s_ps{c}_{i}", tag="ps")
            # the two heads' S^T matmuls use disjoint PE row groups
            # (k-dims at partitions 0-63 vs 64-127) -> run concurrently
            nc.tensor.matmul(
                ps[:, 0:512],
                lhsT=kt_sb[u][0:64, i * 128:(i + 1) * 128],
                rhs=qt_sb[u][0:64, nsl],
                start=True,
                stop=True,
            )
            nc.tensor.matmul(
                ps[:, 512:1024],
                lhsT=kt_sb[u][64:128, i * 128:(i + 1) * 128],
                rhs=qt_sb[u][64:128, nsl],
                start=True,
                stop=True,
            )
            pt = ppool.tile([128, 1024], DT, name=f"pt{c}_{i}", tag="pt")
            nc.scalar.activation(
                out=pt, in_=ps, func=mybir.ActivationFunctionType.Exp
            )
            pts[s] = pt

        def norm_chunk(c):
            u, j = CHUNKS[c]
            nsl = slice(j * 512, (j + 1) * 512)
            po_a, po_b = po_cur.pop(c)
            # row 64 of each po holds the softmax denominators. A [1, 1024]
            # reciprocal would run on ONE DVE lane at ~6 cyc/elem (6.5us!),
            # so bounce the row through DRAM, re-read it spread over 128
            # partitions, recip there (48 cycles), and bounce back out.
            o_un = spool.tile([DH + 1, 1024], F32, name=f"ou{c}", tag="oun")
            nc.vector.tensor_copy(out=o_un[:, 0:512], in_=po_a)
            nc.vector.tensor_copy(out=o_un[:, 512:1024], in_=po_b)
            sd = dpool.tile([1, 1024], F32, name=f"sd{c}", tag="sd")
            nc.sync.dma_start(out=sd, in_=o_un[DH:DH + 1, :])
            r8 = spool.tile([128, 8], F32, name=f"r8{c}", tag="r8")
            sd_8 = bass.AP(tensor=sd.tensor, offset=sd.offset, ap=[[8, 128], [1, 8]])
            nc.sync.dma_start(out=r8, in_=sd_8)
            r8i = spool.tile([128, 8], F32, name=f"r8i{c}", tag="r8i")
            nc.vector.reciprocal(out=r8i, in_=r8)
            rdram = dpool.tile([1, 1024], F32, name=f"rd{c}", tag="rd")
            rd_8 = bass.AP(
                tensor=rdram.tensor, offset=rdram.offset, ap=[[8, 128], [1, 8]]
            )
            nc.sync.dma_start(out=rd_8, in_=r8i)
            # broadcast 1/s across 64 partitions via a partition-stride-0 read
            rbc = spool.tile([64, 1024], F32, name=f"rb{c}", tag="rbc")
            bc_ap = bass.AP(
                tensor=rdram.tensor,
                offset=rdram.offset,
                ap=[[0, 64]] + [list(d) for d in rdram.ap[1:]],
            )
            nc.sync.dma_start(out=rbc, in_=bc_ap)
            nc.vector.tensor_tensor(
                out=onT_sb[u][0:64, nsl],
                in0=o_un[0:DH, 0:512],
                in1=rbc[:, 0:512],
                op=mybir.AluOpType.mult,
            )
            nc.vector.tensor_tensor(
                out=onT_sb[u][64:128, nsl],
                in0=o_un[0:DH, 512:1024],
                in1=rbc[:, 512:1024],
                op=mybir.AluOpType.mult,
            )
            if u == 3:
                # proj for this query chunk's n-tiles is now unblocked;
                # delay a few quanta so the norm chain (DVE+DMA bounce)
                # lands before the first proj matmul reads onT
                fillers.append(gen_delay(4))
                for nt in range(j * 4, j * 4 + 4):
                    fillers.append(gen_proj_group(nt, 0))
                    fillers.append(gen_proj_group(nt, 1))

        def pv(g):
            c, i = divmod(g, 16)
            u, j = CHUNKS[c]
            if i == 0:
                po_cur[c] = (
                    ps_o.tile([DH + 1, 512], F32, name=f"poa{c}", tag="po"),
                    ps_o.tile([DH + 1, 512], F32, name=f"pob{c}", tag="po"),
                )
            po_a, po_b = po_cur[c]
            pt = pts.pop(g)
            nc.tensor.matmul(
                po_a,
                lhsT=vaug_sb[i][:, 2 * u, :],
                rhs=pt[:, 0:512],
                start=(i == 0),
                stop=(i == 15),
            )
            nc.tensor.matmul(
                po_b,
                lhsT=vaug_sb[i][:, 2 * u + 1, :],
                rhs=pt[:, 512:1024],
                start=(i == 0),
                stop=(i == 15),
            )
            if i == 15:
                norm_chunk(c)

        # ---- schedule ----
        # warm the ACT exp table while DMAs stream in, off the critical path
        wrm = spool.tile([1, 8], F32, name="wrm", tag="wrm")
        nc.vector.memset(wrm, 0.0)
        wrm2 = spool.tile([1, 8], F32, name="wrm2", tag="wrm2")
        nc.scalar.activation(
            out=wrm2, in_=wrm, func=mybir.ActivationFunctionType.Exp
        )

        # pre-phase: kt[0] + qt[0] chunk 0 + full V, emitted in DMA-arrival
        # order (xT key-chunk c unlocks both kt[0] chunk c and V m-tiles 4c..)
        qk_group_now(0, 4, ps_s)
        qk_group_now(0, 0, ps_s)
        for mt in range(4):
            v_tile_now(mt, ps_s)
        for jc in range(1, 4):
            qk_group_now(jc, 4, ps_s)
            for mt in range(4 * jc, 4 * jc + 4):
                v_tile_now(mt, ps_s)

        # filler queue: remaining Q^T chunks for pair 0, then K^T/Q^T for
        # later pairs (consumed during earlier pairs' ACT-bound attention)
        for j in range(1, 4):
            fillers.append(gen_qk_group(j, 0))
        for u in range(1, 4):
            for j in range(4):
                fillers.append(gen_qk_group(j, 4 + u))
            for j in range(4):
                fillers.append(gen_qk_group(j, u))

        # pipeline prologue: S/exp run D=2 steps ahead of PV
        s_exp(0)
        s_exp(1)
        for g in range(NSTEP):
            emit_filler()
            pv(g)
            if g + 2 < NSTEP:
                s_exp(g + 2)

        # drain any remaining filler work (tail proj groups)
        while fillers or cur_gen[0] is not None:
            if cur_gen[0] is None:
                cur_gen[0] = fillers.popleft()
            for _ in cur_gen[0]:
                pass
            cur_gen[0] = None

    _replace_sem_range_clear(nc)
    _split_multi_waits(nc)
    return nc


_NC_CACHE = None


def _get_nc():
    global _NC_CACHE
    if _NC_CACHE is None:
        _NC_CACHE = build_bass()
    return _NC_CACHE


def make_in_maps(x, w_qkv, b_qkv, w_proj):
    """Host-side sharding: returns the 8 per-core input dicts."""
    x = np.asarray(x, np.float32)
    w_qkv = np.asarray(w_qkv, np.float32)
    b_qkv = np.asarray(b_qkv, np.float32)
    w_proj = np.asarray(w_proj, np.float32)

    in_maps = []
    for core in range(NCORES):
        b, g = divmod(core, 2)
        cs = slice(512 * g, 512 * g + 512)

        wq = w_qkv[:, 0:1024][:, cs] * SCALE
        wk = w_qkv[:, 1024:2048][:, cs]
        wv_s = w_qkv[:, 2048:3072][:, cs]
        bq = b_qkv[0:1024][cs] * SCALE
        bk = b_qkv[1024:2048][cs]
        bv = b_qkv[2048:3072][cs]

        xT_aug = np.zeros((KT * 128, N), np.float32)
        xT_aug[:C] = x[b].T
        xT_aug[C] = 1.0

        wv_aug = np.zeros((KT * 128, HD), np.float32)
        wv_aug[:C] = wv_s
        wv_aug[C] = bv

        bqk_np = np.concatenate([bq, bk]).reshape(8, 128).T.copy()

        in_maps.append({
            "xT": xT_aug.astype(NPDT),
            "wqk": np.concatenate([wq, wk], axis=1).astype(NPDT),
            "wv": wv_aug.astype(NPDT),
            "bqk": np.ascontiguousarray(bqk_np, np.float32),
            "wp": w_proj[cs, :].astype(NPDT),
        })
    return in_maps


def assemble_output(results, b_proj):
    b_proj = np.asarray(b_proj, np.float32)
    outs = [np.asarray(r["out"], np.float32) for r in results]
    return np.stack([outs[2 * b] + outs[2 * b + 1] + b_proj for b in range(B)])


def run(x, w_qkv, b_qkv, w_proj, b_proj, **spmd_kwargs):
    from concourse.bass_utils import run_bass_kernel_spmd

    nc = _get_nc()
    in_maps = make_in_maps(x, w_qkv, b_qkv, w_proj)
    res = run_bass_kernel_spmd(nc, in_maps, list(range(NCORES)), **spmd_kwargs)
    return assemble_output(res.results, b_proj), res


def kernel(x, w_qkv, b_qkv, w_proj, b_proj):
    out, _ = run(x, w_qkv, b_qkv, w_proj, b_proj)
    return out


# revision 21
# speedup vs baseline: 1.1968x; 1.0240x over previous
"""Multi-head self-attention (B=4, N=2048, C=1024, H=16) on 8 Trainium2 cores.

Sharding: core = (batch b, head-group g) with b in 0..3, g in 0..1.
Each core computes, for its batch and its 8 heads:
    QKV projection -> per-head attention (S^T layout softmax) -> proj partial.
Host sums the two head-group partials per batch and adds b_proj.

v2 schedule: heads are processed in PAIRS (2u, 2u+1). The two S^T matmuls
of a pair use disjoint PE row-groups (k-dims at partitions 0-63 vs 64-127),
so the hardware runs them concurrently (~2x on the QK^T stage). Each
[128 keys, 1024] PSUM tile holds S^T for both heads (512 queries each) and
is consumed by ONE wide exp on ACT - the exp stream is the kernel's
critical path, so everything else (QKV projection groups, proj groups) is
chopped into small "filler quanta" interleaved between attention steps to
soak up the PE slack without stalling ACT.

Device-side layout choices (all transposes done on host, none on device):
  - x is shipped pre-transposed as xT [C, N] (+ a ones row for the V bias).
  - Q^T/K^T are produced as [c', n] tiles directly (lhsT = w_qk natural).
  - V is produced in natural [m, h*d] layout augmented with a ones column per
    head; the ones column makes the PV matmul emit the softmax row-sums.
  - Softmax runs on S^T tiles [m, n]: exp on the scalar engine, sums via the
    V ones-column, normalization via reciprocal + partition-broadcast + mult.
  - Projection consumes O^T [hd, n] tiles directly as lhsT.
"""

import os
import sys

if "/opt/trn_rl_repo" not in sys.path:
    sys.path.insert(0, "/opt/trn_rl_repo")

# the kernel executes through PJRT on the axon-tunneled NeuronCores; a
# cpu-pinned JAX_PLATFORMS (as some harnesses set for the reference) would
# hide the devices — fix it before anything imports jax
if "axon" not in os.environ.get("JAX_PLATFORMS", "axon"):
    os.environ["JAX_PLATFORMS"] = "axon"

from collections import deque
from contextlib import ExitStack

import ml_dtypes
import numpy as np

import concourse.bass as bass
import concourse.tile as tile
from concourse import mybir

B, N, C = 4, 2048, 1024
H, DH = 16, 64
HG = 8                # heads per core
HD = HG * DH          # 512 head-dims per core
SCALE = DH ** -0.5    # 0.125
KT = 9                # contraction k-tiles for V matmul (8 x + 1 bias/ones)
NCORES = 8

F32 = mybir.dt.float32

# matmul operand dtype knob: mybir.dt.bfloat16 or mybir.dt.float32r
DT = mybir.dt.bfloat16
NPDT = ml_dtypes.bfloat16 if DT == mybir.dt.bfloat16 else np.float32


def _replace_sem_range_clear(nc):
    """This walrus build rejects the EVENT_SEMAPHORE_RANGE_CLEAR InstISA that
    TileContext emits at kernel end. Replace it with per-semaphore negative
    sem-inc updates (attached to cheap Pool-engine carriers) that bring every
    kernel semaphore back to zero — equivalent effect, using only encodings
    this compiler accepts. Runs before _split_multi_waits."""
    f = nc.m.functions[0]
    blocks = list(f.blocks)
    snaps = [list(b.instructions) for b in blocks]
    totals = {}
    for insts in snaps:
        for i in insts:
            si = i.sync_info
            if si:
                for u in si.on_update:
                    if u.sync_type == "semaphore":
                        totals[u.id] = totals.get(u.id, 0) + u.update_value
    newlists = []
    for insts in snaps:
        newlist = []
        for i in insts:
            if type(i).__name__ == "InstISA" and "RANGE_CLEAR" in (i.op_name or ""):
                d = i.ant_dict
                for sem in range(d["range_first"], d["range_last"] + 1):
                    v = totals.get(sem, 0)
                    if v == 0:
                        continue
                    car = mybir.InstEventSemaphore(
                        name=nc.get_next_instruction_name()
                    )
                    car.engine = i.engine
                    car.sync_info = mybir.SyncInfo(
                        on_wait=[],
                        on_update=[
                            mybir.SyncUpdate(
                                sync_type="semaphore",
                                id=sem,
                                update_mode="sem-wr-imm",
                                update_value=0,
                                update_reg=None,
                            )
                        ],
                    )
                    newlist.append(car)
                continue  # drop the RANGE_CLEAR itself
            newlist.append(i)
        newlists.append(newlist)
    for b, nl in zip(blocks, newlists):
        b.instructions = nl


def _prune_same_engine_waits(nc):
    """Drop sem-ge waits whose semaphore is only ever incremented by
    instructions on the SAME engine as the waiter: engine streams execute in
    program order, so such waits are satisfied by construction (a kernel
    where they weren't would deadlock). Tile emits these for pool-ring
    accounting; on hardware each one costs a ~40-60ns carrier after
    _split_multi_waits. Runs before _split_multi_waits."""
    f = nc.m.functions[0]
    updaters = {}
    for b in f.blocks:
        for i in b.instructions:
            si = i.sync_info
            if not si:
                continue
            for u in si.on_update:
                if u.sync_type == "semaphore" and u.update_mode == "sem-inc":
                    if (u.update_value or 0) >= 0:
                        updaters.setdefault(u.id, set()).add(str(i.engine))
                    else:
                        updaters.setdefault(u.id, set()).add("<negative>")
                elif u.sync_type == "semaphore":
                    updaters.setdefault(u.id, set()).add("<other-mode>")
    for b in f.blocks:
        for i in b.instructions:
            si = i.sync_info
            if not si or not si.on_wait:
                continue
            kept = [
                w
                for w in si.on_wait
                if not (
                    w.sync_type == "semaphore"
                    and w.wait_mode == "sem-ge-imm"
                    and w.wait_reg is None
                    and updaters.get(w.id) == {str(i.engine)}
                )
            ]
            if len(kept) != len(si.on_wait):
                i.sync_info = mybir.SyncInfo(
                    on_wait=kept, on_update=list(si.on_update)
                )


def _split_multi_waits(nc):
    """Legalize for walrus builds that allow only ONE sync wait per
    instruction: hoist extra waits onto cheap same-engine *real* carrier
    instructions inserted immediately before the offending instruction.
    A wait executed earlier in the same engine stream is strictly more
    conservative, so semantics are preserved.

    For matmuls, walrus encodes the matmul's syncs into its paired
    LDWEIGHTS struct, so the (LDW, MM) pair is treated as having capacity
    for ONE wait total; extras go onto scratch-LDWEIGHTS carriers placed
    before the pair (a stray weight load between complete pairs is
    harmless — every real matmul reloads its own weights)."""
    def make_carrier(engine):
        car = mybir.InstEventSemaphore(name=nc.get_next_instruction_name())
        car.engine = engine
        return car

    f = nc.m.functions[0]
    blocks = list(f.blocks)
    snapshots = [list(b.instructions) for b in blocks]
    newlists = []
    for insts in snapshots:
        newlist = []
        for i in insts:
            si = i.sync_info
            ty = type(i).__name__
            if si is not None and len(si.on_wait) > 1:
                waits = list(si.on_wait)
                is_mm = ty == "InstMatmult"
                # matmul syncs share the paired LDW's single wait slot, which
                # the LDW may already use — keep none on the matmul itself
                keep = 0 if is_mm else 1
                extras = waits[: len(waits) - keep]
                kept = waits[len(waits) - keep:]
                # insertion position: before the paired LDW for matmuls
                pos = len(newlist)
                if is_mm and pos > 0 and type(newlist[-1]).__name__ == "InstLdweights":
                    pos -= 1
                carriers = []
                for w in extras:
                    car = make_carrier(i.engine)
                    if car is None:
                        kept = waits  # cannot split; leave untouched
                        carriers = []
                        break
                    car.sync_info = mybir.SyncInfo(on_wait=[w], on_update=[])
                    carriers.append(car)
                if carriers or len(kept) < len(waits):
                    newlist[pos:pos] = carriers
                    i.sync_info = mybir.SyncInfo(
                        on_wait=kept, on_update=list(si.on_update)
                    )
            newlist.append(i)
        newlists.append(newlist)
    # assigning every block's list also wipes the stray auto-appended carriers
    for b, nl in zip(blocks, newlists):
        b.instructions = nl


def build_bass():
    nc = bass.Bass()

    xT = nc.declare_dram_parameter("xT", [KT * 128, N], DT, isOutput=False)
    wqk = nc.declare_dram_parameter("wqk", [C, 1024], DT, isOutput=False)
    wv = nc.declare_dram_parameter("wv", [KT * 128, HD], DT, isOutput=False)
    bqk = nc.declare_dram_parameter("bqk", [128, 8], F32, isOutput=False)
    wp = nc.declare_dram_parameter("wp", [HD, C], DT, isOutput=False)
    out = nc.declare_dram_parameter("out", [N, C], F32, isOutput=True)

    with tile.TileContext(nc) as tc, ExitStack() as ctx:
        res = ctx.enter_context(tc.tile_pool(name="res", bufs=1))
        ppool = ctx.enter_context(tc.tile_pool(name="ppool", bufs=4))
        spool = ctx.enter_context(tc.tile_pool(name="spool", bufs=2))
        opool = ctx.enter_context(tc.tile_pool(name="opool", bufs=2))
        ps_s = ctx.enter_context(tc.tile_pool(name="ps_s", bufs=2, space="PSUM"))
        ps_o = ctx.enter_context(tc.tile_pool(name="ps_o", bufs=3, space="PSUM"))
        ps_mm = ctx.enter_context(tc.tile_pool(name="ps_mm", bufs=1, space="PSUM"))
        dpool = ctx.enter_context(tc.tile_pool(name="dpool", bufs=4, space="DRAM"))

        # ---- resident SBUF tensors ----
        xT_sb = [res.tile([128, N], DT, name=f"xt{k}", tag=f"xt{k}") for k in range(KT)]
        wqk_sb = [res.tile([128, 1024], DT, name=f"wqk{k}", tag=f"wqk{k}") for k in range(8)]
        wv_sb = [res.tile([128, HD], DT, name=f"wv{k}", tag=f"wv{k}") for k in range(KT)]
        wp_sb = [res.tile([128, C], DT, name=f"wp{t}", tag=f"wp{t}") for t in range(4)]
        bqk_sb = res.tile([128, 8], F32, name="bqk_sb", tag="bqk_sb")
        qt_sb = [res.tile([128, N], DT, name=f"qt{t}", tag=f"qt{t}") for t in range(4)]
        kt_sb = [res.tile([128, N], DT, name=f"kt{t}", tag=f"kt{t}") for t in range(4)]
        vaug_sb = [res.tile([128, HG, DH + 1], DT, name=f"va{m}", tag=f"va{m}") for m in range(16)]
        onT_sb = [res.tile([128, N], DT, name=f"ot{t}", tag=f"ot{t}") for t in range(4)]

        # DMA issue order matches consumption order: bias, then per-k pairs
        # of (wqk kt[0]-slice, xT chunk-0 tile) so the first QK group's
        # matmuls start as soon as each k-tile lands, then qt[0] slices and
        # the V weights (pre-phase), then the rest.
        nc.sync.dma_start(out=bqk_sb, in_=bqk[:, :])
        for k in range(8):
            nc.sync.dma_start(
                out=wqk_sb[k][:, 512:640], in_=wqk[k * 128:(k + 1) * 128, 512:640]
            )
            nc.sync.dma_start(
                out=xT_sb[k][:, 0:512], in_=xT[k * 128:(k + 1) * 128, 0:512]
            )
        for k in range(8):
            nc.sync.dma_start(
                out=wqk_sb[k][:, 0:128], in_=wqk[k * 128:(k + 1) * 128, 0:128]
            )
        nc.sync.dma_start(out=xT_sb[8][:, 0:512], in_=xT[8 * 128:9 * 128, 0:512])
        for k in range(KT):
            nc.sync.dma_start(out=wv_sb[k], in_=wv[k * 128:(k + 1) * 128, :])
        for j in range(1, 4):
            for k in range(KT):
                nc.sync.dma_start(
                    out=xT_sb[k][:, j * 512:(j + 1) * 512],
                    in_=xT[k * 128:(k + 1) * 128, j * 512:(j + 1) * 512],
                )
        for k in range(8):
            nc.sync.dma_start(
                out=wqk_sb[k][:, 128:512], in_=wqk[k * 128:(k + 1) * 128, 128:512]
            )
            nc.sync.dma_start(
                out=wqk_sb[k][:, 640:1024], in_=wqk[k * 128:(k + 1) * 128, 640:1024]
            )
        for t in range(4):
            nc.sync.dma_start(out=wp_sb[t], in_=wp[t * 128:(t + 1) * 128, :])

        # ---- phase builders ----
        def qk_mms(ps, j, ct, k0, k1):
            for k in range(k0, k1):
                nc.tensor.matmul(
                    ps,
                    lhsT=wqk_sb[k][:, ct * 128:(ct + 1) * 128],
                    rhs=xT_sb[k][:, j * 512:(j + 1) * 512],
                    start=(k == 0),
                    stop=(k == 7),
                )

        def qk_copy(ps, j, ct):
            dst = qt_sb[ct] if ct < 4 else kt_sb[ct - 4]
            nc.vector.tensor_scalar_add(
                out=dst[:, j * 512:(j + 1) * 512],
                in0=ps,
                scalar1=bqk_sb[:, ct:ct + 1],
            )

        def qk_group_now(j, ct, pool):
            # pre-phase version: borrow the (otherwise idle) ps_s slots
            ps = pool.tile([128, 1024], F32, name=f"qkn{ct}_{j}", tag="ps")
            qk_mms(ps[:, 0:512], j, ct, 0, 8)
            qk_copy(ps[:, 0:512], j, ct)

        def gen_qk_group(j, ct):
            # filler version: small quanta on the 1-bank ps_mm pool
            ps = ps_mm.tile([128, 512], F32, name=f"qkg{ct}_{j}", tag="mm")
            for k0 in range(0, 8, 2):
                qk_mms(ps, j, ct, k0, k0 + 2)
                yield
            qk_copy(ps, j, ct)
            yield

        def v_tile_now(mt, pool):
            # V_aug [m, h, d|1] = x @ w_v (+ b_v via ones row)
            ps = pool.tile([128, 1024], F32, name=f"v_ps{mt}", tag="ps")
            for k in range(KT):
                nc.tensor.matmul(
                    ps[:, 0:512],
                    lhsT=xT_sb[k][:, mt * 128:(mt + 1) * 128],
                    rhs=wv_sb[k],
                    start=(k == 0),
                    stop=(k == KT - 1),
                )
            va = vaug_sb[mt]
            nc.vector.memset(va[:, :, DH:DH + 1], 1.0)
            nc.vector.tensor_copy(
                out=va[:, :, 0:DH],
                in_=ps[:, 0:512].rearrange("p (h d) -> p h d", h=HG),
            )

        def gen_proj_group(nt, cc, pool, tag):
            # one (n-tile, c-chunk) group of the proj partial; each half is
            # DMA'd out as soon as its copy lands so the output trickles out
            # instead of piling into the kernel tail
            ob = opool.tile([128, 512], F32, name=f"ob{nt}_{cc}", tag="ob")
            py = pool.tile([128, 512], F32, name=f"y_ps{nt}_{cc}", tag=tag)
            for t in range(4):
                nc.tensor.matmul(
                    py,
                    lhsT=onT_sb[t][:, nt * 128:(nt + 1) * 128],
                    rhs=wp_sb[t][:, cc * 512:(cc + 1) * 512],
                    start=(t == 0),
                    stop=(t == 3),
                )
                if t == 1:
                    yield
            nc.vector.tensor_copy(out=ob, in_=py)
            nc.sync.dma_start(
                out=out[nt * 128:(nt + 1) * 128, cc * 512:(cc + 1) * 512], in_=ob
            )
            yield

        def gen_delay(n):
            for _ in range(n):
                yield

        # ---- filler machinery: one quantum (~0.2-0.4us of PE) per step ----
        # queue holds descriptors; generators are materialized lazily so the
        # tail drain can re-target proj psum to the (then-free) ps_s banks
        fillers = deque()
        cur_gen = [None]

        def make_gen(item, pool, tag):
            kind = item[0]
            if kind == "qk":
                return gen_qk_group(item[1], item[2])
            if kind == "proj":
                return gen_proj_group(item[1], item[2], pool, tag)
            return gen_delay(item[1])

        def emit_filler():
            while True:
                if cur_gen[0] is None:
                    if not fillers:
                        return
                    cur_gen[0] = make_gen(fillers.popleft(), ps_mm, "mm")
                try:
                    next(cur_gen[0])
                    return
                except StopIteration:
                    cur_gen[0] = None

        # ---- attention: one global software pipeline over all chunks ----
        # chunk c = (pair u, query chunk j); pairs 0/1 run pair-outer, then
        # pairs 2/3 interleave per query chunk so each query chunk's proj
        # (which needs ALL pairs' onT) unblocks early and spreads across the
        # attention instead of piling into the tail. 16 key-steps per chunk;
        # the S/exp stream runs D steps ahead of the PV stream so chunk
        # boundaries never bubble the ACT exp stream.
        CHUNKS = (
            [(u, j) for u in range(2) for j in range(4)]
            + [(u, j) for j in range(4) for u in (2, 3)]
        )
        NSTEP = len(CHUNKS) * 16
        pts = {}
        po_cur = {}

        def s_exp(s):
            c, i = divmod(s, 16)
            u, j = CHUNKS[c]
            nsl = slice(j * 512, (j + 1) * 512)
            ps = ps_s.tile([128, 1024], F32, name=f"s_ps{c}_{i}", tag="ps")
            # the two heads' S^T matmuls use disjoint PE row groups
            # (k-dims at partitions 0-63 vs 64-127) -> run concurrently
            nc.tensor.matmul(
                ps[:, 0:512],
                lhsT=kt_sb[u][0:64, i * 128:(i + 1) * 128],
                rhs=qt_sb[u][0:64, nsl],
                start=True,
                stop=True,
            )
            nc.tensor.matmul(
                ps[:, 512:1024],
                lhsT=kt_sb[u][64:128, i * 128:(i + 1) * 128],
                rhs=qt_sb[u][64:128, nsl],
                start=True,
                stop=True,
            )
            pt = ppool.tile([128, 1024], DT, name=f"pt{c}_{i}", tag="pt")
            nc.scalar.activation(
                out=pt, in_=ps, func=mybir.ActivationFunctionType.Exp
            )
            pts[s] = pt

        def norm_chunk(c):
            u, j = CHUNKS[c]
            nsl = slice(j * 512, (j + 1) * 512)
            po_a, po_b = po_cur.pop(c)
            # row 64 of each po holds the softmax denominators. A [1, 1024]
            # reciprocal would run on ONE DVE lane at ~6 cyc/elem (6.5us!),
            # so bounce the row through DRAM, re-read it spread over 128
            # partitions, recip there (48 cycles), and bounce back out.
            o_un = spool.tile([DH + 1, 1024], F32, name=f"ou{c}", tag="oun")
            nc.vector.tensor_copy(out=o_un[:, 0:512], in_=po_a)
            nc.vector.tensor_copy(out=o_un[:, 512:1024], in_=po_b)
            sd = dpool.tile([1, 1024], F32, name=f"sd{c}", tag="sd")
            nc.sync.dma_start(out=sd, in_=o_un[DH:DH + 1, :])
            r8 = spool.tile([128, 8], F32, name=f"r8{c}", tag="r8")
            sd_8 = bass.AP(tensor=sd.tensor, offset=sd.offset, ap=[[8, 128], [1, 8]])
            nc.sync.dma_start(out=r8, in_=sd_8)
            r8i = spool.tile([128, 8], F32, name=f"r8i{c}", tag="r8i")
            nc.vector.reciprocal(out=r8i, in_=r8)
            rdram = dpool.tile([1, 1024], F32, name=f"rd{c}", tag="rd")
            rd_8 = bass.AP(
                tensor=rdram.tensor, offset=rdram.offset, ap=[[8, 128], [1, 8]]
            )
            nc.sync.dma_start(out=rd_8, in_=r8i)
            # broadcast 1/s across 64 partitions via a partition-stride-0 read
            rbc = spool.tile([64, 1024], F32, name=f"rb{c}", tag="rbc")
            bc_ap = bass.AP(
                tensor=rdram.tensor,
                offset=rdram.offset,
                ap=[[0, 64]] + [list(d) for d in rdram.ap[1:]],
            )
            nc.sync.dma_start(out=rbc, in_=bc_ap)
            nc.vector.tensor_tensor(
                out=onT_sb[u][0:64, nsl],
                in0=o_un[0:DH, 0:512],
                in1=rbc[:, 0:512],
                op=mybir.AluOpType.mult,
            )
            nc.vector.tensor_tensor(
                out=onT_sb[u][64:128, nsl],
                in0=o_un[0:DH, 512:1024],
                in1=rbc[:, 512:1024],
                op=mybir.AluOpType.mult,
            )
            if u == 3:
                # proj for this query chunk's n-tiles is now unblocked;
                # delay a few quanta so the norm chain (DVE+DMA bounce)
                # lands before the first proj matmul reads onT
                fillers.append(("delay", 4))
                for nt in range(j * 4, j * 4 + 4):
                    fillers.append(("proj", nt, 0))
                    fillers.append(("proj", nt, 1))

        def pv(g):
            c, i = divmod(g, 16)
            u, j = CHUNKS[c]
            if i == 0:
                po_cur[c] = (
                    ps_o.tile([DH + 1, 512], F32, name=f"poa{c}", tag="po"),
                    ps_o.tile([DH + 1, 512], F32, name=f"pob{c}", tag="po"),
                )
            po_a, po_b = po_cur[c]
            pt = pts.pop(g)
            nc.tensor.matmul(
                po_a,
                lhsT=vaug_sb[i][:, 2 * u, :],
                rhs=pt[:, 0:512],
                start=(i == 0),
                stop=(i == 15),
            )
            nc.tensor.matmul(
                po_b,
                lhsT=vaug_sb[i][:, 2 * u + 1, :],
                rhs=pt[:, 512:1024],
                start=(i == 0),
                stop=(i == 15),
            )
            if i == 15:
                norm_chunk(c)

        # ---- schedule ----
        # warm the ACT exp table while DMAs stream in, off the critical path
        wrm = spool.tile([1, 8], F32, name="wrm", tag="wrm")
        nc.vector.memset(wrm, 0.0)
        wrm2 = spool.tile([1, 8], F32, name="wrm2", tag="wrm2")
        nc.scalar.activation(
            out=wrm2, in_=wrm, func=mybir.ActivationFunctionType.Exp
        )

        # pre-phase: kt[0] + qt[0] chunk 0 + full V, emitted in DMA-arrival
        # order (xT key-chunk c unlocks both kt[0] chunk c and V m-tiles 4c..)
        qk_group_now(0, 4, ps_s)
        qk_group_now(0, 0, ps_s)
        for mt in range(4):
            v_tile_now(mt, ps_s)
        for jc in range(1, 4):
            qk_group_now(jc, 4, ps_s)
            for mt in range(4 * jc, 4 * jc + 4):
                v_tile_now(mt, ps_s)

        # filler queue: remaining Q^T chunks for pair 0, then K^T/Q^T for
        # later pairs ordered by first-use (pairs 2/3 interleave per query
        # chunk at the end, so their K^T and first Q^T chunks come early)
        for j in range(1, 4):
            fillers.append(("qk", j, 0))
        for j in range(4):
            fillers.append(("qk", j, 5))
        for j in range(4):
            fillers.append(("qk", j, 1))
        for j in range(4):
            fillers.append(("qk", j, 6))
        fillers.append(("qk", 0, 2))
        for j in range(4):
            fillers.append(("qk", j, 7))
        fillers.append(("qk", 0, 3))
        for j in range(1, 4):
            fillers.append(("qk", j, 2))
            fillers.append(("qk", j, 3))

        # pipeline prologue: S/exp run D=2 steps ahead of PV
        s_exp(0)
        s_exp(1)
        for g in range(NSTEP):
            emit_filler()
            pv(g)
            if g + 2 < NSTEP:
                s_exp(g + 2)

        # drain remaining filler work (tail proj groups). Attention is done,
        # so the ps_s banks are free - run the drain double-buffered there
        # instead of serializing on the single ps_mm bank.
        if cur_gen[0] is not None:
            for _ in cur_gen[0]:
                pass
            cur_gen[0] = None
        while fillers:
            item = fillers.popleft()
            for _ in make_gen(item, ps_s, "ps"):
                pass

    _prune_same_engine_waits(nc)
    _replace_sem_range_clear(nc)
    _split_multi_waits(nc)
    return nc


_NC_CACHE = None


def _get_nc():
    global _NC_CACHE
    if _NC_CACHE is None:
        _NC_CACHE = build_bass()
    return _NC_CACHE


def make_in_maps(x, w_qkv, b_qkv, w_proj):
    """Host-side sharding: returns the 8 per-core input dicts."""
    x = np.asarray(x, np.float32)
    w_qkv = np.asarray(w_qkv, np.float32)
    b_qkv = np.asarray(b_qkv, np.float32)
    w_proj = np.asarray(w_proj, np.float32)

    in_maps = []
    for core in range(NCORES):
        b, g = divmod(core, 2)
        cs = slice(512 * g, 512 * g + 512)

        wq = w_qkv[:, 0:1024][:, cs] * SCALE
        wk = w_qkv[:, 1024:2048][:, cs]
        wv_s = w_qkv[:, 2048:3072][:, cs]
        bq = b_qkv[0:1024][cs] * SCALE
        bk = b_qkv[1024:2048][cs]
        bv = b_qkv[2048:3072][cs]

        xT_aug = np.zeros((KT * 128, N), np.float32)
        xT_aug[:C] = x[b].T
        xT_aug[C] = 1.0

        wv_aug = np.zeros((KT * 128, HD), np.float32)
        wv_aug[:C] = wv_s
        wv_aug[C] = bv

        bqk_np = np.concatenate([bq, bk]).reshape(8, 128).T.copy()

        in_maps.append({
            "xT": xT_aug.astype(NPDT),
            "wqk": np.concatenate([wq, wk], axis=1).astype(NPDT),
            "wv": wv_aug.astype(NPDT),
            "bqk": np.ascontiguousarray(bqk_np, np.float32),
            "wp": w_proj[cs, :].astype(NPDT),
        })
    return in_maps


def assemble_output(results, b_proj):
    b_proj = np.asarray(b_proj, np.float32)
    outs = [np.asarray(r["out"], np.float32) for r in results]
    return np.stack([outs[2 * b] + outs[2 * b + 1] + b_proj for b in range(B)])


def run(x, w_qkv, b_qkv, w_proj, b_proj, **spmd_kwargs):
    from concourse.bass_utils import run_bass_kernel_spmd

    nc = _get_nc()
    in_maps = make_in_maps(x, w_qkv, b_qkv, w_proj)
    res = run_bass_kernel_spmd(nc, in_maps, list(range(NCORES)), **spmd_kwargs)
    return assemble_output(res.results, b_proj), res


def kernel(x, w_qkv, b_qkv, w_proj, b_proj):
    out, _ = run(x, w_qkv, b_qkv, w_proj, b_proj)
    return out
